# revision 7
# baseline (speedup 1.0000x reference)
"""Trainium2 Bass kernel for nn_MessagePassing (gnn_message_passing).

Math (reference semantics; fc biases dropped -- BatchNorm cancels them exactly):
  NodeUpdate:
    z1 = mask * ([node_i ; edge] @ W_node)
    nbr_sum = sum_j sigmoid(bn1(z1)_g) * tanh(bn1(z1)_e)
    un = tanh(node + bn2(nbr_sum))
  EdgeUpdate:
    node_j = gather(un, nbr_idx)
    z2 = (mask * un_i * node_j) @ W2
    two_body = sigmoid(bn2b(z2)_g) * tanh(bn2b(z2)_e)
    z3[j,k'] = T[j] + P[k'] for k' != j, where W3 = [W3a;W3b;W3c;W3d;W3e],
      T = W3a'un_i + W3b'node_j + W3d'edge,  P = W3c'node_j + W3e'edge
    three_body = sum_{k' != j} sigmoid(bn3b(z3)_g)*tanh(bn3b(z3)_e)
               = (sum over ALL k') - (k'=j diagonal term)
    ue = tanh(edge + two_body + bnsum(three_body))

Distribution: 8 cores = (B=4) x (At halves). NodeUpdate replicated within each
batch pair (BN stats are duplication invariant under equal counts). Exact
training-mode BN via 4 small AllGathers of per-core [mean, mean-square].

Layout: feature-major (channels on partitions). Host pre-transposes inputs,
post-transposes outputs, and builds a one-hot matrix so the neighbor gather is
a TensorE matmul. Atom axis rotated per-core so own atoms are local 0..127
(identical SPMD program on all cores).
"""

import numpy as np

B, At, Nbr, F = 4, 256, 16, 128
NCORES = 8
AH = At // 2
RN = At * Nbr
RE = AH * Nbr
EPS = 1e-5
NCHUNK = 512

_NC_CACHE = None


def _build():
    import concourse.bacc as bacc
    import concourse.mybir as mybir
    import concourse.tile as tile

    dt = mybir.dt.float32
    bf = mybir.dt.float16
    AF = mybir.ActivationFunctionType
    AL = mybir.AluOpType
    X = mybir.AxisListType.X

    nc = bacc.Bacc("TRN2", target_bir_lowering=False, debug=False,
                   enable_asserts=True, num_devices=NCORES)

    edge_d = nc.dram_tensor("edge_fm", (F, RN), dt, kind="ExternalInput")
    node_d = nc.dram_tensor("node_fm", (F, At), dt, kind="ExternalInput")
    maskb_d = nc.dram_tensor("mask_b", (F, RN), dt, kind="ExternalInput")
    gmat_d = nc.dram_tensor("gmat", (At, RE), dt, kind="ExternalInput")
    wnode_d = nc.dram_tensor("w_node", (2 * F, 2 * F), dt, kind="ExternalInput")
    w2_d = nc.dram_tensor("w2", (F, 2 * F), dt, kind="ExternalInput")
    w3_d = nc.dram_tensor("w3", (5 * F, 2 * F), dt, kind="ExternalInput")
    id_d = nc.dram_tensor("id128", (F, F), dt, kind="ExternalInput")
    pdefs = [("bn1g", 2), ("bn1b", 2), ("bn2g", 1), ("bn2b", 1), ("bn2bg", 2),
             ("bn2bb", 2), ("bn3g", 2), ("bn3b", 2), ("bnsg", 1), ("bnsb", 1)]
    pdram = {nm: nc.dram_tensor(nm, (F, k), dt, kind="ExternalInput")
             for nm, k in pdefs}

    un_o = nc.dram_tensor("un_out", (F, At), dt, kind="ExternalOutput")
    ue_o = nc.dram_tensor("ue_out", (F, RE), dt, kind="ExternalOutput")

    with tile.TileContext(nc) as tc:
        with tc.tile_pool(name="const", bufs=1) as cst, \
             tc.tile_pool(name="dram", bufs=1, space="DRAM") as dr, \
             tc.tile_pool(name="psum", bufs=3, space="PSUM") as ps, \
             tc.tile_pool(name="small", bufs=1) as sm:

            # ---- persistent loads ----
            edge = cst.tile([F, RN], dt)
            for c in range(4):
                nc.sync.dma_start(edge[:, c * 1024:(c + 1) * 1024],
                                  edge_d.ap()[:, c * 1024:(c + 1) * 1024])
            node = cst.tile([F, At], dt)
            nc.sync.dma_start(node[:], node_d.ap())
            id128 = cst.tile([F, F], dt)
            nc.sync.dma_start(id128[:], id_d.ap())
            wn = [[cst.tile([F, F], dt, name=f"wn{k}{m}") for m in range(2)]
                  for k in range(2)]
            for k in range(2):
                for m in range(2):
                    nc.sync.dma_start(
                        wn[k][m][:],
                        wnode_d.ap()[k * F:(k + 1) * F, m * F:(m + 1) * F])
            w2 = [cst.tile([F, F], dt, name=f"w2{m}") for m in range(2)]
            for m in range(2):
                nc.sync.dma_start(w2[m][:], w2_d.ap()[:, m * F:(m + 1) * F])
            w3 = {}
            for bi_, blk in enumerate("abcde"):
                for m in range(2):
                    t = cst.tile([F, F], dt, name=f"w3{blk}{m}")
                    nc.sync.dma_start(
                        t[:], w3_d.ap()[bi_ * F:(bi_ + 1) * F, m * F:(m + 1) * F])
                    w3[blk, m] = t
            params = {}
            for nm, k in pdefs:
                t = sm.tile([F, k], dt, name=nm + "_t")
                nc.sync.dma_start(t[:], pdram[nm].ap())
                params[nm] = t
            un = sm.tile([F, At], dt, name="un")
            nbrs = sm.tile([F, At], dt, name="nbrs")

            warm = sm.tile([F, 2], dt, name="warm")
            nc.vector.memset(warm[:], 0.0)
            nc.scalar.activation(warm[:, 0:1], warm[:, 1:2], AF.Sigmoid)

            # ---- helpers ----
            def meansq(dst2, mv):
                nc.vector.tensor_copy(dst2[:, 0:1], mv[:, 0:1])
                nc.vector.tensor_mul(dst2[:, 1:2], mv[:, 0:1], mv[:, 0:1])
                nc.vector.tensor_add(dst2[:, 1:2], dst2[:, 1:2], mv[:, 1:2])

            sync_ctr = [0]

            def sync_avg(payload, K):
                i = sync_ctr[0]
                sync_ctr[0] += 1
                cin = dr.tile([F * K], dt, name=f"cin{i}")
                cout = dr.tile([NCORES * F * K], dt, addr_space="Shared",
                               name=f"cout{i}")
                nc.sync.dma_start(cin[:].rearrange("(c k) -> c k", k=K),
                                  payload[:])
                nc.gpsimd.collective_compute(
                    "AllGather", AL.bypass,
                    replica_groups=[list(range(NCORES))],
                    ins=[cin.opt()], outs=[cout.opt()])
                g = sm.tile([F, K, NCORES], dt, name=f"g{i}")
                nc.sync.dma_start(
                    g[:], cout[:].rearrange("(r c k) -> c k r", c=F, k=K))
                avg = sm.tile([F, K], dt, name=f"avg{i}")
                nc.vector.tensor_reduce(avg[:], g[:], axis=X, op=AL.add)
                nc.vector.tensor_scalar(avg[:], avg[:], 1.0 / NCORES, None,
                                        op0=AL.mult)
                return avg

            def affine(avg2, g_ap, b_ap, out_ab, col):
                i = sync_ctr[0]
                mean, msq = avg2[:, 0:1], avg2[:, 1:2]
                var = sm.tile([F, 1], dt, name=f"var{i}_{col}")
                nc.vector.tensor_mul(var[:], mean, mean)
                nc.vector.tensor_sub(var[:], msq, var[:])
                nc.vector.tensor_scalar(var[:], var[:], EPS, None, op0=AL.add)
                nc.vector.reciprocal(var[:], var[:])
                nc.scalar.sqrt(var[:], var[:])
                al = out_ab[:, col:col + 1]
                de = out_ab[:, col + 1:col + 2]
                nc.vector.tensor_mul(al, var[:], g_ap)
                nc.vector.tensor_mul(de, mean, al)
                nc.vector.tensor_sub(de, b_ap, de)

            def bn_local(tiles, nch, name):
                pay = sm.tile([F, 2 * len(tiles)], dt, name=name + "_pay")
                for m, t in enumerate(tiles):
                    st = sm.tile([F, nch * 6], dt, name=f"{name}_st{m}")
                    for ch in range(nch):
                        nc.vector.bn_stats(st[:, ch * 6:(ch + 1) * 6],
                                           t[:, ch * 512:(ch + 1) * 512])
                    mv = sm.tile([F, 2], dt, name=f"{name}_mv{m}")
                    nc.vector.bn_aggr(mv[:], st[:])
                    meansq(pay[:, 2 * m:2 * m + 2], mv)
                return pay

            # long-lived mid tensors (allocated before p1 for stack order)
            pL = tc.alloc_tile_pool(name="pL", bufs=1)
            # ---- window 1: mask + gather matrix (until c2 built) ----
            p1 = tc.alloc_tile_pool(name="p1", bufs=1)
            maskb = p1.tile([F, RN], dt)
            for c in range(4):
                nc.sync.dma_start(maskb[:, c * 1024:(c + 1) * 1024],
                                  maskb_d.ap()[:, c * 1024:(c + 1) * 1024])
            gmat0 = p1.tile([F, RE], dt)
            nc.sync.dma_start(gmat0[:], gmat_d.ap()[0:F, :])
            gmat1 = p1.tile([F, RE], dt)
            nc.sync.dma_start(gmat1[:], gmat_d.ap()[F:At, :])

            # ================= Stage A: NodeUpdate =================
            pA = tc.alloc_tile_pool(name="pA", bufs=1)
            tz1_0 = pA.tile([F, RN], dt, tag="tzA")
            tz1_1 = pA.tile([F, RN], dt, tag="tzB")
            tz1 = [tz1_0, tz1_1]
            for m in range(2):
                for ch in range(RN // NCHUNK):
                    sl = slice(ch * NCHUNK, (ch + 1) * NCHUNK)
                    pz = ps.tile([F, NCHUNK], dt, tag="mm")
                    nb = node[:, ch * 32:(ch + 1) * 32].unsqueeze(2) \
                        .broadcast_to((F, 32, Nbr))
                    nc.tensor.matmul(pz[:], wn[0][m][:], nb,
                                     start=True, stop=False)
                    nc.tensor.matmul(pz[:], wn[1][m][:], edge[:, sl],
                                     start=False, stop=True)
                    nc.vector.tensor_mul(tz1[m][:, sl], pz[:], maskb[:, sl])

            pay1 = bn_local(tz1, RN // NCHUNK, "bn1")
            avg1 = sync_avg(pay1, 4)
            ab1 = sm.tile([F, 4], dt, name="ab1")
            affine(avg1[:, 0:2], params["bn1g"][:, 0:1], params["bn1b"][:, 0:1],
                   ab1, 0)
            affine(avg1[:, 2:4], params["bn1g"][:, 1:2], params["bn1b"][:, 1:2],
                   ab1, 2)

            sg = pA.tile([F, RN], dt, tag="sgA")
            nc.scalar.activation(sg[:], tz1_0[:], AF.Sigmoid,
                                 bias=ab1[:, 1:2], scale=ab1[:, 0:1])
            th = pA.tile([F, RN], dt, tag="tzA")   # reuses tz1_0 slot
            nc.scalar.activation(th[:], tz1_1[:], AF.Tanh,
                                 bias=ab1[:, 3:4], scale=ab1[:, 2:3])
            nc.vector.tensor_mul(sg[:], sg[:], th[:])
            nc.vector.tensor_reduce(
                nbrs[:], sg[:].rearrange("p (a j) -> p a j", j=Nbr),
                axis=X, op=AL.add)
            pA.release()

            st2 = sm.tile([F, 6], dt, name="st2")
            nc.vector.bn_stats(st2[:], nbrs[:])
            mv2 = sm.tile([F, 2], dt, name="mv2")
            nc.vector.bn_aggr(mv2[:], st2[:])
            pay2 = sm.tile([F, 2], dt, name="pay2")
            meansq(pay2[:], mv2)
            avg2 = sync_avg(pay2, 2)
            ab2 = sm.tile([F, 2], dt, name="ab2")
            affine(avg2, params["bn2g"][:, 0:1], params["bn2b"][:, 0:1], ab2, 0)

            nc.vector.tensor_scalar(un[:], nbrs[:], ab2[:, 0:1], ab2[:, 1:2],
                                    op0=AL.mult, op1=AL.add)
            nc.vector.tensor_add(un[:], un[:], node[:])
            nc.scalar.activation(un[:], un[:], AF.Tanh)
            nc.sync.dma_start(un_o.ap(), un[:])

            # ================= Stage C: gather, z2, T, P =================
            un_am = [pL.tile([F, F], dt, name=f"un_am{t}") for t in range(2)]
            for t in range(2):
                pt = ps.tile([F, F], dt, tag="tr")
                nc.tensor.transpose(pt[:], un[:, t * F:(t + 1) * F], id128[:])
                nc.scalar.copy(un_am[t][:], pt[:])

            node_j = pL.tile([F, RE], dt)
            for ch in range(RE // NCHUNK):
                sl = slice(ch * NCHUNK, (ch + 1) * NCHUNK)
                pg = ps.tile([F, NCHUNK], dt, tag="mm")
                nc.tensor.matmul(pg[:], un_am[0][:], gmat0[:, sl],
                                 start=True, stop=False)
                nc.tensor.matmul(pg[:], un_am[1][:], gmat1[:, sl],
                                 start=False, stop=True)
                nc.scalar.copy(node_j[:, sl], pg[:])

            c2 = pL.tile([F, RE], dt)
            un_own_b = un[:, 0:AH].unsqueeze(2).broadcast_to((F, AH, Nbr))
            nc.vector.tensor_mul(
                c2[:].rearrange("p (a j) -> p a j", j=Nbr),
                node_j[:].rearrange("p (a j) -> p a j", j=Nbr), un_own_b)
            nc.vector.tensor_mul(c2[:], c2[:], maskb[:, 0:RE])
            p1.release()

            z2 = [pL.tile([F, RE], bf, name=f"z2_{m}") for m in range(2)]
            for m in range(2):
                for ch in range(RE // NCHUNK):
                    sl = slice(ch * NCHUNK, (ch + 1) * NCHUNK)
                    pz = ps.tile([F, NCHUNK], dt, tag="mm")
                    nc.tensor.matmul(pz[:], w2[m][:], c2[:, sl])
                    nc.scalar.copy(z2[m][:, sl], pz[:])

            Tb = [pL.tile([F, RE], bf, name=f"Tb{m}") for m in range(2)]
            Pb = [pL.tile([F, RE], bf, name=f"Pb{m}") for m in range(2)]
            for m in range(2):
                for ch in range(RE // NCHUNK):
                    sl = slice(ch * NCHUNK, (ch + 1) * NCHUNK)
                    ub = un[:, ch * 32:(ch + 1) * 32].unsqueeze(2) \
                        .broadcast_to((F, 32, Nbr))
                    pt_ = ps.tile([F, NCHUNK], dt, tag="mm")
                    nc.tensor.matmul(pt_[:], w3["b", m][:], node_j[:, sl],
                                     start=True, stop=False)
                    nc.tensor.matmul(pt_[:], w3["d", m][:], edge[:, sl],
                                     start=False, stop=False)
                    nc.tensor.matmul(pt_[:], w3["a", m][:], ub,
                                     start=False, stop=True)
                    nc.scalar.copy(Tb[m][:, sl], pt_[:])
                    pp = ps.tile([F, NCHUNK], dt, tag="mm")
                    nc.tensor.matmul(pp[:], w3["c", m][:], node_j[:, sl],
                                     start=True, stop=False)
                    nc.tensor.matmul(pp[:], w3["e", m][:], edge[:, sl],
                                     start=False, stop=True)
                    nc.scalar.copy(Pb[m][:, sl], pp[:])

            D = [pL.tile([F, RE], bf, name=f"D{m}") for m in range(2)]
            sd2 = sm.tile([F, 2], dt, name="sd2")
            sqscr = pL.tile([F, RE], bf)
            for m in range(2):
                nc.vector.tensor_add(D[m][:], Tb[m][:], Pb[m][:])
                nc.scalar.activation(sqscr[:], D[m][:], AF.Square,
                                     accum_out=sd2[:, m:m + 1])

            cross = sm.tile([F, 2], dt, name="cross")
            for m in range(2):
                rT = sm.tile([F, AH], dt, name=f"rT{m}")
                nc.vector.tensor_reduce(
                    rT[:], Tb[m][:].rearrange("p (a j) -> p a j", j=Nbr),
                    axis=X, op=AL.add)
                rP = sm.tile([F, AH], dt, name=f"rP{m}")
                nc.vector.tensor_reduce(
                    rP[:], Pb[m][:].rearrange("p (a j) -> p a j", j=Nbr),
                    axis=X, op=AL.add)
                nc.vector.tensor_mul(rT[:], rT[:], rP[:])
                nc.vector.tensor_reduce(cross[:, m:m + 1], rT[:],
                                        axis=X, op=AL.add)

            payT = bn_local(Tb, RE // NCHUNK, "bnT")
            payP = bn_local(Pb, RE // NCHUNK, "bnP")
            pay2b = bn_local(z2, RE // NCHUNK, "bn2b")

            pay3 = sm.tile([F, 4], dt, name="pay3")
            n3 = float(RE * (Nbr - 1))
            for m in range(2):
                sT = sm.tile([F, 2], dt, name=f"sT{m}")
                nc.vector.tensor_scalar(sT[:], payT[:, 2 * m:2 * m + 2],
                                        float(RE), None, op0=AL.mult)
                sP = sm.tile([F, 2], dt, name=f"sP{m}")
                nc.vector.tensor_scalar(sP[:], payP[:, 2 * m:2 * m + 2],
                                        float(RE), None, op0=AL.mult)
                nc.vector.tensor_add(pay3[:, 2 * m:2 * m + 1],
                                     sT[:, 0:1], sP[:, 0:1])
                nc.vector.tensor_scalar(pay3[:, 2 * m:2 * m + 1],
                                        pay3[:, 2 * m:2 * m + 1],
                                        float(Nbr - 1) / n3, None, op0=AL.mult)
                s2 = sm.tile([F, 1], dt, name=f"s2_{m}")
                nc.vector.tensor_add(s2[:], sT[:, 1:2], sP[:, 1:2])
                nc.vector.tensor_scalar(s2[:], s2[:], float(Nbr), None,
                                        op0=AL.mult)
                cr2 = sm.tile([F, 1], dt, name=f"cr2_{m}")
                nc.vector.tensor_scalar(cr2[:], cross[:, m:m + 1], 2.0, None,
                                        op0=AL.mult)
                nc.vector.tensor_add(s2[:], s2[:], cr2[:])
                nc.vector.tensor_sub(s2[:], s2[:], sd2[:, m:m + 1])
                nc.vector.tensor_scalar(pay3[:, 2 * m + 1:2 * m + 2], s2[:],
                                        1.0 / n3, None, op0=AL.mult)

            pay23 = sm.tile([F, 8], dt, name="pay23")
            nc.vector.tensor_copy(pay23[:, 0:4], pay2b[:])
            nc.vector.tensor_copy(pay23[:, 4:8], pay3[:])
            avg23 = sync_avg(pay23, 8)
            ab2b = sm.tile([F, 4], dt, name="ab2b")
            affine(avg23[:, 0:2], params["bn2bg"][:, 0:1],
                   params["bn2bb"][:, 0:1], ab2b, 0)
            affine(avg23[:, 2:4], params["bn2bg"][:, 1:2],
                   params["bn2bb"][:, 1:2], ab2b, 2)
            ab3 = sm.tile([F, 4], dt, name="ab3")
            affine(avg23[:, 4:6], params["bn3g"][:, 0:1],
                   params["bn3b"][:, 0:1], ab3, 0)
            affine(avg23[:, 6:8], params["bn3g"][:, 1:2],
                   params["bn3b"][:, 1:2], ab3, 2)

            # two_body
            tb = pL.tile([F, RE], dt)
            sg2 = pL.tile([F, RE], bf)
            nc.scalar.activation(sg2[:], z2[0][:], AF.Sigmoid,
                                 bias=ab2b[:, 1:2], scale=ab2b[:, 0:1])
            nc.scalar.activation(tb[:], z2[1][:], AF.Tanh,
                                 bias=ab2b[:, 3:4], scale=ab2b[:, 2:3])
            nc.vector.tensor_mul(tb[:], tb[:], sg2[:])


            # ================= Stage E: three-body grid =================
            three = pL.tile([F, RE], dt)
            with tc.tile_pool(name="grid", bufs=3) as pG:
                for j in range(Nbr):
                    gg = pG.tile([F, RE], bf, tag="gg")
                    tsl = Tb[0][:].rearrange("p (a j) -> p a j", j=Nbr)[
                        :, :, j:j + 1].broadcast_to((F, AH, Nbr))
                    nc.vector.tensor_add(
                        gg[:].rearrange("p (a j) -> p a j", j=Nbr),
                        Pb[0][:].rearrange("p (a j) -> p a j", j=Nbr), tsl)
                    sgj = pG.tile([F, RE], bf, tag="sgj")
                    nc.scalar.activation(sgj[:], gg[:], AF.Sigmoid,
                                         bias=ab3[:, 1:2], scale=ab3[:, 0:1])
                    ge = pG.tile([F, RE], bf, tag="ge")
                    tsl_e = Tb[1][:].rearrange("p (a j) -> p a j", j=Nbr)[
                        :, :, j:j + 1].broadcast_to((F, AH, Nbr))
                    nc.vector.tensor_add(
                        ge[:].rearrange("p (a j) -> p a j", j=Nbr),
                        Pb[1][:].rearrange("p (a j) -> p a j", j=Nbr), tsl_e)
                    thj = pG.tile([F, RE], bf, tag="thj")
                    nc.scalar.activation(thj[:], ge[:], AF.Tanh,
                                         bias=ab3[:, 3:4], scale=ab3[:, 2:3])
                    prj = pG.tile([F, RE], bf, tag="prj")
                    nc.vector.tensor_mul(prj[:], sgj[:], thj[:])
                    nc.vector.tensor_reduce(
                        three[:].rearrange("p (a j) -> p a j", j=Nbr)[:, :, j],
                        prj[:].rearrange("p (a k) -> p a k", k=Nbr),
                        axis=X, op=AL.add)

                dsg = pG.tile([F, RE], bf, tag="sgj")
                nc.scalar.activation(dsg[:], D[0][:], AF.Sigmoid,
                                     bias=ab3[:, 1:2], scale=ab3[:, 0:1])
                dth = pG.tile([F, RE], bf, tag="thj")
                nc.scalar.activation(dth[:], D[1][:], AF.Tanh,
                                     bias=ab3[:, 3:4], scale=ab3[:, 2:3])
                dpr = pG.tile([F, RE], bf, tag="prj")
                nc.vector.tensor_mul(dpr[:], dsg[:], dth[:])
                nc.vector.tensor_sub(three[:], three[:], dpr[:])

            st4 = sm.tile([F, 4 * 6], dt, name="st4")
            for ch in range(RE // NCHUNK):
                nc.vector.bn_stats(st4[:, ch * 6:(ch + 1) * 6],
                                   three[:, ch * 512:(ch + 1) * 512])
            mv4 = sm.tile([F, 2], dt, name="mv4")
            nc.vector.bn_aggr(mv4[:], st4[:])
            pay4 = sm.tile([F, 2], dt, name="pay4")
            meansq(pay4[:], mv4)
            avg4 = sync_avg(pay4, 2)
            ab4 = sm.tile([F, 2], dt, name="ab4")
            affine(avg4, params["bnsg"][:, 0:1], params["bnsb"][:, 0:1], ab4, 0)

            ue = pL.tile([F, RE], dt)
            nc.vector.tensor_scalar(ue[:], three[:], ab4[:, 0:1], ab4[:, 1:2],
                                    op0=AL.mult, op1=AL.add)
            nc.vector.tensor_add(ue[:], ue[:], tb[:])
            nc.vector.tensor_add(ue[:], ue[:], edge[:, 0:RE])
            nc.scalar.activation(ue[:], ue[:], AF.Tanh)
            nc.sync.dma_start(ue_o.ap(), ue[:])

            pL.release()

    nc.compile()
    return nc


def _prep_core(inputs, core):
    b, h = core // 2, core % 2
    roll = lambda arr, ax: np.roll(arr, -h * AH, axis=ax)

    edge = roll(np.asarray(inputs["edge_embedding"][b]), 0)
    node = roll(np.asarray(inputs["node_embedding"][b]), 0)
    mask = roll(np.asarray(inputs["nbr_mask"][b]), 0)
    idx = roll(np.asarray(inputs["nbr_idx"][b]), 0)
    idx_local = (idx - h * AH) % At

    edge_fm = np.ascontiguousarray(edge.reshape(RN, F).T)
    node_fm = np.ascontiguousarray(node.T)
    mask_b = np.broadcast_to(
        mask.reshape(1, RN).astype(np.float32), (F, RN)).copy()
    gmat = np.zeros((At, RE), np.float32)
    gmat[idx_local[:AH].reshape(RE), np.arange(RE)] = 1.0

    pk2 = lambda p: np.ascontiguousarray(
        np.asarray(p, np.float32).reshape(2, F).T)
    pk1 = lambda p: np.ascontiguousarray(
        np.asarray(p, np.float32).reshape(F, 1))
    return {
        "edge_fm": edge_fm, "node_fm": node_fm, "mask_b": mask_b, "gmat": gmat,
        "w_node": np.asarray(inputs["W_node"], np.float32),
        "w2": np.asarray(inputs["W2"], np.float32),
        "w3": np.asarray(inputs["W3"], np.float32),
        "id128": np.eye(F, dtype=np.float32),
        "bn1g": pk2(inputs["g_bn1"]), "bn1b": pk2(inputs["be_bn1"]),
        "bn2g": pk1(inputs["g_bn2"]), "bn2b": pk1(inputs["be_bn2"]),
        "bn2bg": pk2(inputs["g_bn2b"]), "bn2bb": pk2(inputs["be_bn2b"]),
        "bn3g": pk2(inputs["g_bn3b"]), "bn3b": pk2(inputs["be_bn3b"]),
        "bnsg": pk1(inputs["g_bnsum"]), "bnsb": pk1(inputs["be_bnsum"]),
    }


def kernel(**inputs):
    global _NC_CACHE
    import concourse.bass_utils as bass_utils
    if _NC_CACHE is None:
        _NC_CACHE = _build()
    nc = _NC_CACHE

    in_maps = [_prep_core(inputs, i) for i in range(NCORES)]
    res = bass_utils.run_bass_kernel_spmd(
        nc, in_maps, core_ids=list(range(NCORES)))
    results = res.results

    un = np.empty((B, At, F), np.float32)
    ue = np.empty((B, At, Nbr, F), np.float32)
    for b in range(B):
        un[b] = results[2 * b]["un_out"].T
        for h in range(2):
            r = results[2 * b + h]["ue_out"]
            ue[b, h * AH:(h + 1) * AH] = (
                r.reshape(F, AH, Nbr).transpose(1, 2, 0))
    return un, ue


# revision 10
# speedup vs baseline: 1.1245x; 1.1245x over previous
"""Trainium2 Bass kernel for nn_MessagePassing (gnn_message_passing).

Math (reference semantics; fc biases dropped -- BatchNorm cancels them exactly):
  NodeUpdate:
    z1 = mask * ([node_i ; edge] @ W_node)
    nbr_sum = sum_j sigmoid(bn1(z1)_g) * tanh(bn1(z1)_e)
    un = tanh(node + bn2(nbr_sum))
  EdgeUpdate:
    node_j = gather(un, nbr_idx)  (one-hot matmul)
    z2 = (mask * un_i * node_j) @ W2
    two_body = sigmoid(bn2b(z2)_g) * tanh(bn2b(z2)_e)
    z3[j,k'] = T[j] + P[k'] (k' != j), W3 row-split into a..e:
      T = W3a'un_i + W3b'node_j + W3d'edge ; P = W3c'node_j + W3e'edge
    three_body = sum_{k'!=j} sig(bn3b(z3)_g)*tanh(bn3b(z3)_e)
               = (sum over ALL k') - (k'=j diagonal)
    ue = tanh(edge + two_body + bnsum(three_body))

Distribution: 8 cores = (B=4) x (At halves); NodeUpdate replicated per pair.
Exact BN via small AllGathers of per-core [mean, mean-square] (equal counts).
Feature-major layout; host transposes I/O and builds the one-hot gather.
Matmuls run in float32r (full-rate fp32 PE mode, ~1e-4); the three-body grid
runs in fp16 on GpSimd (adds) + ScalarE (acts) + VectorE (mult/reduce).
rsqrt for BN affines is Newton-on-DVE so ScalarE keeps one act table set.
"""

import numpy as np

B, At, Nbr, F = 4, 256, 16, 128
NCORES = 8
AH = At // 2
RN = At * Nbr
RE = AH * Nbr
EPS = 1e-5
NCHUNK = 512

_NC_CACHE = None


def _build():
    import concourse.bacc as bacc
    import concourse.mybir as mybir
    import concourse.tile as tile

    dt = mybir.dt.float32
    fr = mybir.dt.float32r
    f16 = mybir.dt.float16
    i32 = mybir.dt.int32
    AF = mybir.ActivationFunctionType
    AL = mybir.AluOpType
    X = mybir.AxisListType.X

    nc = bacc.Bacc("TRN2", target_bir_lowering=False, debug=False,
                   enable_asserts=True, num_devices=NCORES)

    edge_d = nc.dram_tensor("edge_fm", (F, RN), fr, kind="ExternalInput")
    node_d = nc.dram_tensor("node_fm", (F, At), fr, kind="ExternalInput")
    maskb_d = nc.dram_tensor("mask_b", (F, RN), dt, kind="ExternalInput")
    gmat_d = nc.dram_tensor("gmat", (At, RE), fr, kind="ExternalInput")
    wnode_d = nc.dram_tensor("w_node", (2 * F, 2 * F), fr, kind="ExternalInput")
    w2_d = nc.dram_tensor("w2", (F, 2 * F), fr, kind="ExternalInput")
    w3_d = nc.dram_tensor("w3", (5 * F, 2 * F), fr, kind="ExternalInput")
    id_d = nc.dram_tensor("id128", (F, F), dt, kind="ExternalInput")
    pdefs = [("bn1g", 2), ("bn1b", 2), ("bn2g", 1), ("bn2b", 1), ("bn2bg", 2),
             ("bn2bb", 2), ("bn3g", 2), ("bn3b", 2), ("bnsg", 1), ("bnsb", 1)]
    pdram = {nm: nc.dram_tensor(nm, (F, k), dt, kind="ExternalInput")
             for nm, k in pdefs}

    un_o = nc.dram_tensor("un_out", (F, At), dt, kind="ExternalOutput")
    ue_o = nc.dram_tensor("ue_out", (F, RE), dt, kind="ExternalOutput")

    with tile.TileContext(nc) as tc:
        with tc.tile_pool(name="const", bufs=1) as cst, \
             tc.tile_pool(name="dram", bufs=1, space="DRAM") as dr, \
             tc.tile_pool(name="psum", bufs=3, space="PSUM") as ps, \
             tc.tile_pool(name="small", bufs=1) as sm:

            # ---- persistent loads ----
            edge = cst.tile([F, RN], fr)
            for c in range(4):
                nc.sync.dma_start(edge[:, c * 1024:(c + 1) * 1024],
                                  edge_d.ap()[:, c * 1024:(c + 1) * 1024])
            edge32 = edge[:].bitcast(dt)
            node = cst.tile([F, At], fr)
            nc.sync.dma_start(node[:], node_d.ap())
            id128 = cst.tile([F, F], dt)
            nc.sync.dma_start(id128[:], id_d.ap())
            wn = [[cst.tile([F, F], fr, name=f"wn{k}{m}") for m in range(2)]
                  for k in range(2)]
            for k in range(2):
                for m in range(2):
                    nc.sync.dma_start(
                        wn[k][m][:],
                        wnode_d.ap()[k * F:(k + 1) * F, m * F:(m + 1) * F])
            w2 = [cst.tile([F, F], fr, name=f"w2{m}") for m in range(2)]
            for m in range(2):
                nc.sync.dma_start(w2[m][:], w2_d.ap()[:, m * F:(m + 1) * F])
            w3 = {}
            for bi_, blk in enumerate("abcde"):
                for m in range(2):
                    t = cst.tile([F, F], fr, name=f"w3{blk}{m}")
                    nc.sync.dma_start(
                        t[:], w3_d.ap()[bi_ * F:(bi_ + 1) * F, m * F:(m + 1) * F])
                    w3[blk, m] = t
            params = {}
            for nm, k in pdefs:
                t = sm.tile([F, k], dt, name=nm + "_t")
                nc.sync.dma_start(t[:], pdram[nm].ap())
                params[nm] = t
            un = sm.tile([F, At], fr, name="un")
            un_tmp = sm.tile([F, At], dt, name="un_tmp")
            nbrs = sm.tile([F, At], dt, name="nbrs")
            magic = sm.tile([F, 1], i32, name="magic")
            nc.vector.memset(magic[:], 0x5f3759df)
            one_i = sm.tile([F, 1], i32, name="one_i")
            nc.vector.memset(one_i[:], 1)

            warm = sm.tile([F, 2], dt, name="warm")
            nc.vector.memset(warm[:], 0.0)
            nc.scalar.activation(warm[:, 0:1], warm[:, 1:2], AF.Sigmoid)

            # ---- helpers ----
            def meansq(dst2, mv):
                nc.vector.tensor_copy(dst2[:, 0:1], mv[:, 0:1])
                nc.vector.tensor_mul(dst2[:, 1:2], mv[:, 0:1], mv[:, 0:1])
                nc.vector.tensor_add(dst2[:, 1:2], dst2[:, 1:2], mv[:, 1:2])

            sync_ctr = [0]

            def sync_avg(payload, K):
                i = sync_ctr[0]
                sync_ctr[0] += 1
                cin = dr.tile([F * K], dt, name=f"cin{i}")
                cout = dr.tile([NCORES * F * K], dt, addr_space="Shared",
                               name=f"cout{i}")
                nc.sync.dma_start(cin[:].rearrange("(c k) -> c k", k=K),
                                  payload[:])
                nc.gpsimd.collective_compute(
                    "AllGather", AL.bypass,
                    replica_groups=[list(range(NCORES))],
                    ins=[cin.opt()], outs=[cout.opt()])
                g = sm.tile([F, K, NCORES], dt, name=f"g{i}")
                nc.sync.dma_start(
                    g[:], cout[:].rearrange("(r c k) -> c k r", c=F, k=K))
                avg = sm.tile([F, K], dt, name=f"avg{i}")
                nc.vector.tensor_reduce(avg[:], g[:], axis=X, op=AL.add)
                nc.vector.tensor_scalar(avg[:], avg[:], 1.0 / NCORES, None,
                                        op0=AL.mult)
                return avg

            def affine_group(avg, gammas, betas, G, name):
                """avg (F, 2G) = interleaved [mean, msq] pairs.
                gammas/betas: list of G (F,1) APs.
                Returns ab (F, 2G) interleaved [alpha, delta] pairs.
                rsqrt is Newton iteration on DVE (keeps ACT table set fixed).
                """
                av = avg[:].rearrange("p (g two) -> p g two", two=2)
                mean = av[:, :, 0]          # (F, G) strided
                msq = av[:, :, 1]
                var = sm.tile([F, G], dt, name=name + "_var")
                nc.vector.tensor_mul(var[:], mean, mean)
                nc.vector.tensor_sub(var[:], msq, var[:])
                nc.vector.tensor_scalar(var[:], var[:], EPS, None, op0=AL.add)
                # Newton rsqrt: y0 from bit trick, 3 iterations
                y = sm.tile([F, G], dt, name=name + "_y")
                yi = y[:].bitcast(i32)
                nc.vector.tensor_tensor(
                    yi, var[:].bitcast(i32),
                    one_i[:, 0:1].broadcast_to((F, G)),
                    op=AL.logical_shift_right)
                nc.vector.tensor_tensor(
                    yi, magic[:, 0:1].broadcast_to((F, G)), yi,
                    op=AL.subtract)
                t = sm.tile([F, G], dt, name=name + "_t")
                for _ in range(3):
                    nc.vector.tensor_mul(t[:], y[:], y[:])
                    nc.vector.tensor_mul(t[:], t[:], var[:])
                    nc.vector.tensor_scalar(t[:], t[:], -0.5, 1.5,
                                            op0=AL.mult, op1=AL.add)
                    nc.vector.tensor_mul(y[:], y[:], t[:])
                ab = sm.tile([F, 2 * G], dt, name=name + "_ab")
                abv = ab[:].rearrange("p (g two) -> p g two", two=2)
                al, de = abv[:, :, 0], abv[:, :, 1]
                gam = sm.tile([F, G], dt, name=name + "_gam")
                bet = sm.tile([F, G], dt, name=name + "_bet")
                for gi_ in range(G):
                    nc.vector.tensor_copy(gam[:, gi_:gi_ + 1], gammas[gi_])
                    nc.vector.tensor_copy(bet[:, gi_:gi_ + 1], betas[gi_])
                nc.vector.tensor_mul(al, y[:], gam[:])
                nc.vector.tensor_mul(de, mean, al)
                nc.vector.tensor_sub(de, bet[:], de)
                return ab

            def bn_local(tiles, nch, name):
                pay = sm.tile([F, 2 * len(tiles)], dt, name=name + "_pay")
                for m, t in enumerate(tiles):
                    st = sm.tile([F, nch * 6], dt, name=f"{name}_st{m}")
                    for ch in range(nch):
                        nc.vector.bn_stats(st[:, ch * 6:(ch + 1) * 6],
                                           t[:, ch * 512:(ch + 1) * 512])
                    mv = sm.tile([F, 2], dt, name=f"{name}_mv{m}")
                    nc.vector.bn_aggr(mv[:], st[:])
                    meansq(pay[:, 2 * m:2 * m + 2], mv)
                return pay

            # long-lived pool first (stack order: pL under p1/pA)
            pL = tc.alloc_tile_pool(name="pL", bufs=1)
            # ---- window 1: mask + gather matrix ----
            p1 = tc.alloc_tile_pool(name="p1", bufs=1)
            maskb = p1.tile([F, RN], dt)
            for c in range(4):
                nc.sync.dma_start(maskb[:, c * 1024:(c + 1) * 1024],
                                  maskb_d.ap()[:, c * 1024:(c + 1) * 1024])
            gmat0 = p1.tile([F, RE], fr)
            nc.sync.dma_start(gmat0[:], gmat_d.ap()[0:F, :])
            gmat1 = p1.tile([F, RE], fr)
            nc.sync.dma_start(gmat1[:], gmat_d.ap()[F:At, :])

            # ================= Stage A: NodeUpdate =================
            pA = tc.alloc_tile_pool(name="pA", bufs=1)
            tz1_0 = pA.tile([F, RN], dt, tag="tzA")
            tz1_1 = pA.tile([F, RN], dt, tag="tzB")
            tz1 = [tz1_0, tz1_1]
            for m in range(2):
                for ch in range(RN // NCHUNK):
                    sl = slice(ch * NCHUNK, (ch + 1) * NCHUNK)
                    pz = ps.tile([F, NCHUNK], dt, tag="mm")
                    nb = node[:, ch * 32:(ch + 1) * 32].unsqueeze(2) \
                        .broadcast_to((F, 32, Nbr))
                    nc.tensor.matmul(pz[:], wn[0][m][:], nb,
                                     start=True, stop=False)
                    nc.tensor.matmul(pz[:], wn[1][m][:], edge[:, sl],
                                     start=False, stop=True)
                    nc.vector.tensor_mul(tz1[m][:, sl], pz[:], maskb[:, sl])

            pay1 = bn_local(tz1, RN // NCHUNK, "bn1")
            avg1 = sync_avg(pay1, 4)
            ab1 = affine_group(
                avg1,
                [params["bn1g"][:, 0:1], params["bn1g"][:, 1:2]],
                [params["bn1b"][:, 0:1], params["bn1b"][:, 1:2]], 2, "af1")

            sg = pA.tile([F, RN], dt, tag="sgA")
            nc.scalar.activation(sg[:], tz1_0[:], AF.Sigmoid,
                                 bias=ab1[:, 1:2], scale=ab1[:, 0:1])
            th = pA.tile([F, RN], dt, tag="tzA")   # reuses tz1_0 slot
            nc.scalar.activation(th[:], tz1_1[:], AF.Tanh,
                                 bias=ab1[:, 3:4], scale=ab1[:, 2:3])
            nc.vector.tensor_mul(sg[:], sg[:], th[:])
            nc.vector.tensor_reduce(
                nbrs[:], sg[:].rearrange("p (a j) -> p a j", j=Nbr),
                axis=X, op=AL.add)
            pA.release()

            st2 = sm.tile([F, 6], dt, name="st2")
            nc.vector.bn_stats(st2[:], nbrs[:])
            mv2 = sm.tile([F, 2], dt, name="mv2")
            nc.vector.bn_aggr(mv2[:], st2[:])
            pay2 = sm.tile([F, 2], dt, name="pay2")
            meansq(pay2[:], mv2)
            avg2 = sync_avg(pay2, 2)
            ab2 = affine_group(avg2, [params["bn2g"][:, 0:1]],
                               [params["bn2b"][:, 0:1]], 1, "af2")

            nc.vector.tensor_scalar(un_tmp[:], nbrs[:], ab2[:, 0:1],
                                    ab2[:, 1:2], op0=AL.mult, op1=AL.add)
            nc.vector.tensor_add(un_tmp[:], un_tmp[:], node[:].bitcast(dt))
            nc.scalar.activation(un[:], un_tmp[:], AF.Tanh)
            nc.sync.dma_start(un_o.ap(), un[:].bitcast(dt))
            un_fr = un[:]
            un32 = un[:].bitcast(dt)

            # ================= Stage C: gather, z2, T, P =================
            un_am = [pL.tile([F, F], fr, name=f"un_am{t}") for t in range(2)]
            for t in range(2):
                pt = ps.tile([F, F], dt, tag="tr")
                nc.tensor.transpose(pt[:], un32[:, t * F:(t + 1) * F], id128[:])
                nc.scalar.copy(un_am[t][:], pt[:])

            node_j = pL.tile([F, RE], fr)
            for ch in range(RE // NCHUNK):
                sl = slice(ch * NCHUNK, (ch + 1) * NCHUNK)
                pg = ps.tile([F, NCHUNK], dt, tag="mm")
                nc.tensor.matmul(pg[:], un_am[0][:], gmat0[:, sl],
                                 start=True, stop=False)
                nc.tensor.matmul(pg[:], un_am[1][:], gmat1[:, sl],
                                 start=False, stop=True)
                nc.scalar.copy(node_j[:, sl], pg[:])

            c2 = pL.tile([F, RE], fr)
            c2tmp = pL.tile([F, RE], dt)
            un_own_b = un32[:, 0:AH].unsqueeze(2).broadcast_to((F, AH, Nbr))
            nc.vector.tensor_mul(
                c2tmp[:].rearrange("p (a j) -> p a j", j=Nbr),
                node_j[:].bitcast(dt).rearrange("p (a j) -> p a j", j=Nbr),
                un_own_b)
            nc.vector.tensor_mul(c2[:], c2tmp[:], maskb[:, 0:RE])
            p1.release()

            # z2 + its stats + early bn2b sync
            z2 = [pL.tile([F, RE], f16, name=f"z2_{m}") for m in range(2)]
            for m in range(2):
                for ch in range(RE // NCHUNK):
                    sl = slice(ch * NCHUNK, (ch + 1) * NCHUNK)
                    pz = ps.tile([F, NCHUNK], dt, tag="mm")
                    nc.tensor.matmul(pz[:], w2[m][:], c2[:, sl])
                    nc.scalar.copy(z2[m][:, sl], pz[:])
            pay2b = bn_local(z2, RE // NCHUNK, "bn2b")
            avg2b = sync_avg(pay2b, 4)
            ab2b = affine_group(
                avg2b,
                [params["bn2bg"][:, 0:1], params["bn2bg"][:, 1:2]],
                [params["bn2bb"][:, 0:1], params["bn2bb"][:, 1:2]], 2, "af2b")

            # T and P (PSUM-accumulated, evicted to fp16)
            Tb = [pL.tile([F, RE], f16, name=f"Tb{m}") for m in range(2)]
            Pb = [pL.tile([F, RE], f16, name=f"Pb{m}") for m in range(2)]
            for m in range(2):
                for ch in range(RE // NCHUNK):
                    sl = slice(ch * NCHUNK, (ch + 1) * NCHUNK)
                    ub = un_fr[:, ch * 32:(ch + 1) * 32].unsqueeze(2) \
                        .broadcast_to((F, 32, Nbr))
                    pt_ = ps.tile([F, NCHUNK], dt, tag="mm")
                    nc.tensor.matmul(pt_[:], w3["b", m][:], node_j[:, sl],
                                     start=True, stop=False)
                    nc.tensor.matmul(pt_[:], w3["d", m][:], edge[:, sl],
                                     start=False, stop=False)
                    nc.tensor.matmul(pt_[:], w3["a", m][:], ub,
                                     start=False, stop=True)
                    nc.scalar.copy(Tb[m][:, sl], pt_[:])
                    pp = ps.tile([F, NCHUNK], dt, tag="mm")
                    nc.tensor.matmul(pp[:], w3["c", m][:], node_j[:, sl],
                                     start=True, stop=False)
                    nc.tensor.matmul(pp[:], w3["e", m][:], edge[:, sl],
                                     start=False, stop=True)
                    nc.scalar.copy(Pb[m][:, sl], pp[:])

            # two_body (needs only ab2b; overlaps T/P assembly)
            tb = pL.tile([F, RE], dt)
            sg2 = pL.tile([F, RE], f16)
            nc.scalar.activation(sg2[:], z2[0][:], AF.Sigmoid,
                                 bias=ab2b[:, 1:2], scale=ab2b[:, 0:1])
            nc.scalar.activation(tb[:], z2[1][:], AF.Tanh,
                                 bias=ab2b[:, 3:4], scale=ab2b[:, 2:3])
            nc.vector.tensor_mul(tb[:], tb[:], sg2[:])

            # D = T + P (fp16, on GpSimd), sumD2 via Square+accum
            D = [pL.tile([F, RE], f16, name=f"D{m}") for m in range(2)]
            sd2 = sm.tile([F, 2], dt, name="sd2")
            sqscr = pL.tile([F, RE], f16)
            for m in range(2):
                nc.gpsimd.tensor_add(D[m][:], Tb[m][:], Pb[m][:])
                nc.scalar.activation(sqscr[:], D[m][:], AF.Square,
                                     accum_out=sd2[:, m:m + 1])

            cross = sm.tile([F, 2], dt, name="cross")
            for m in range(2):
                rT = sm.tile([F, AH], dt, name=f"rT{m}")
                nc.vector.tensor_reduce(
                    rT[:], Tb[m][:].rearrange("p (a j) -> p a j", j=Nbr),
                    axis=X, op=AL.add)
                rP = sm.tile([F, AH], dt, name=f"rP{m}")
                nc.vector.tensor_reduce(
                    rP[:], Pb[m][:].rearrange("p (a j) -> p a j", j=Nbr),
                    axis=X, op=AL.add)
                nc.vector.tensor_mul(rT[:], rT[:], rP[:])
                nc.vector.tensor_reduce(cross[:, m:m + 1], rT[:],
                                        axis=X, op=AL.add)

            payT = bn_local(Tb, RE // NCHUNK, "bnT")
            payP = bn_local(Pb, RE // NCHUNK, "bnP")

            pay3 = sm.tile([F, 4], dt, name="pay3")
            n3 = float(RE * (Nbr - 1))
            for m in range(2):
                sT = sm.tile([F, 2], dt, name=f"sT{m}")
                nc.vector.tensor_scalar(sT[:], payT[:, 2 * m:2 * m + 2],
                                        float(RE), None, op0=AL.mult)
                sP = sm.tile([F, 2], dt, name=f"sP{m}")
                nc.vector.tensor_scalar(sP[:], payP[:, 2 * m:2 * m + 2],
                                        float(RE), None, op0=AL.mult)
                nc.vector.tensor_add(pay3[:, 2 * m:2 * m + 1],
                                     sT[:, 0:1], sP[:, 0:1])
                nc.vector.tensor_scalar(pay3[:, 2 * m:2 * m + 1],
                                        pay3[:, 2 * m:2 * m + 1],
                                        float(Nbr - 1) / n3, None, op0=AL.mult)
                s2 = sm.tile([F, 1], dt, name=f"s2_{m}")
                nc.vector.tensor_add(s2[:], sT[:, 1:2], sP[:, 1:2])
                nc.vector.tensor_scalar(s2[:], s2[:], float(Nbr), None,
                                        op0=AL.mult)
                cr2 = sm.tile([F, 1], dt, name=f"cr2_{m}")
                nc.vector.tensor_scalar(cr2[:], cross[:, m:m + 1], 2.0, None,
                                        op0=AL.mult)
                nc.vector.tensor_add(s2[:], s2[:], cr2[:])
                nc.vector.tensor_sub(s2[:], s2[:], sd2[:, m:m + 1])
                nc.vector.tensor_scalar(pay3[:, 2 * m + 1:2 * m + 2], s2[:],
                                        1.0 / n3, None, op0=AL.mult)

            avg3 = sync_avg(pay3, 4)
            ab3 = affine_group(
                avg3,
                [params["bn3g"][:, 0:1], params["bn3g"][:, 1:2]],
                [params["bn3b"][:, 0:1], params["bn3b"][:, 1:2]], 2, "af3")

            # ================= Stage E: three-body grid =================
            three = pL.tile([F, RE], dt)
            with tc.tile_pool(name="grid", bufs=3) as pG:
                for j in range(Nbr):
                    gg = pG.tile([F, RE], f16, tag="gg")
                    tsl = Tb[0][:].rearrange("p (a j) -> p a j", j=Nbr)[
                        :, :, j:j + 1].broadcast_to((F, AH, Nbr))
                    nc.gpsimd.tensor_add(
                        gg[:].rearrange("p (a j) -> p a j", j=Nbr),
                        Pb[0][:].rearrange("p (a j) -> p a j", j=Nbr), tsl)
                    sgj = pG.tile([F, RE], f16, tag="sgj")
                    nc.scalar.activation(sgj[:], gg[:], AF.Sigmoid,
                                         bias=ab3[:, 1:2], scale=ab3[:, 0:1])
                    ge = pG.tile([F, RE], f16, tag="ge")
                    tsl_e = Tb[1][:].rearrange("p (a j) -> p a j", j=Nbr)[
                        :, :, j:j + 1].broadcast_to((F, AH, Nbr))
                    nc.gpsimd.tensor_add(
                        ge[:].rearrange("p (a j) -> p a j", j=Nbr),
                        Pb[1][:].rearrange("p (a j) -> p a j", j=Nbr), tsl_e)
                    thj = pG.tile([F, RE], f16, tag="thj")
                    nc.scalar.activation(thj[:], ge[:], AF.Tanh,
                                         bias=ab3[:, 3:4], scale=ab3[:, 2:3])
                    prj = pG.tile([F, RE], f16, tag="prj")
                    nc.vector.tensor_mul(prj[:], sgj[:], thj[:])
                    nc.vector.tensor_reduce(
                        three[:].rearrange("p (a j) -> p a j", j=Nbr)[:, :, j],
                        prj[:].rearrange("p (a k) -> p a k", k=Nbr),
                        axis=X, op=AL.add)

                dsg = pG.tile([F, RE], f16, tag="sgj")
                nc.scalar.activation(dsg[:], D[0][:], AF.Sigmoid,
                                     bias=ab3[:, 1:2], scale=ab3[:, 0:1])
                dth = pG.tile([F, RE], f16, tag="thj")
                nc.scalar.activation(dth[:], D[1][:], AF.Tanh,
                                     bias=ab3[:, 3:4], scale=ab3[:, 2:3])
                dpr = pG.tile([F, RE], f16, tag="prj")
                nc.vector.tensor_mul(dpr[:], dsg[:], dth[:])
                nc.vector.tensor_sub(three[:], three[:], dpr[:])

            st4 = sm.tile([F, 4 * 6], dt, name="st4")
            for ch in range(RE // NCHUNK):
                nc.vector.bn_stats(st4[:, ch * 6:(ch + 1) * 6],
                                   three[:, ch * 512:(ch + 1) * 512])
            mv4 = sm.tile([F, 2], dt, name="mv4")
            nc.vector.bn_aggr(mv4[:], st4[:])
            pay4 = sm.tile([F, 2], dt, name="pay4")
            meansq(pay4[:], mv4)
            avg4 = sync_avg(pay4, 2)
            ab4 = affine_group(avg4, [params["bnsg"][:, 0:1]],
                               [params["bnsb"][:, 0:1]], 1, "af4")

            ue = pL.tile([F, RE], dt)
            nc.vector.tensor_scalar(ue[:], three[:], ab4[:, 0:1], ab4[:, 1:2],
                                    op0=AL.mult, op1=AL.add)
            nc.vector.tensor_add(ue[:], ue[:], tb[:])
            nc.vector.tensor_add(ue[:], ue[:], edge32[:, 0:RE])
            nc.scalar.activation(ue[:], ue[:], AF.Tanh)
            nc.sync.dma_start(ue_o.ap(), ue[:])

            pL.release()

    nc.compile()
    return nc


def _prep_core(inputs, core):
    b, h = core // 2, core % 2
    roll = lambda arr, ax: np.roll(arr, -h * AH, axis=ax)

    edge = roll(np.asarray(inputs["edge_embedding"][b]), 0)
    node = roll(np.asarray(inputs["node_embedding"][b]), 0)
    mask = roll(np.asarray(inputs["nbr_mask"][b]), 0)
    idx = roll(np.asarray(inputs["nbr_idx"][b]), 0)
    idx_local = (idx - h * AH) % At

    edge_fm = np.ascontiguousarray(edge.reshape(RN, F).T)
    node_fm = np.ascontiguousarray(node.T)
    mask_b = np.broadcast_to(
        mask.reshape(1, RN).astype(np.float32), (F, RN)).copy()
    gmat = np.zeros((At, RE), np.float32)
    gmat[idx_local[:AH].reshape(RE), np.arange(RE)] = 1.0

    pk2 = lambda p: np.ascontiguousarray(
        np.asarray(p, np.float32).reshape(2, F).T)
    pk1 = lambda p: np.ascontiguousarray(
        np.asarray(p, np.float32).reshape(F, 1))
    return {
        "edge_fm": edge_fm, "node_fm": node_fm, "mask_b": mask_b, "gmat": gmat,
        "w_node": np.asarray(inputs["W_node"], np.float32),
        "w2": np.asarray(inputs["W2"], np.float32),
        "w3": np.asarray(inputs["W3"], np.float32),
        "id128": np.eye(F, dtype=np.float32),
        "bn1g": pk2(inputs["g_bn1"]), "bn1b": pk2(inputs["be_bn1"]),
        "bn2g": pk1(inputs["g_bn2"]), "bn2b": pk1(inputs["be_bn2"]),
        "bn2bg": pk2(inputs["g_bn2b"]), "bn2bb": pk2(inputs["be_bn2b"]),
        "bn3g": pk2(inputs["g_bn3b"]), "bn3b": pk2(inputs["be_bn3b"]),
        "bnsg": pk1(inputs["g_bnsum"]), "bnsb": pk1(inputs["be_bnsum"]),
    }


def kernel(**inputs):
    global _NC_CACHE
    import concourse.bass_utils as bass_utils
    if _NC_CACHE is None:
        _NC_CACHE = _build()
    nc = _NC_CACHE

    in_maps = [_prep_core(inputs, i) for i in range(NCORES)]
    res = bass_utils.run_bass_kernel_spmd(
        nc, in_maps, core_ids=list(range(NCORES)))
    results = res.results

    un = np.empty((B, At, F), np.float32)
    ue = np.empty((B, At, Nbr, F), np.float32)
    for b in range(B):
        un[b] = results[2 * b]["un_out"].T
        for h in range(2):
            r = results[2 * b + h]["ue_out"]
            ue[b, h * AH:(h + 1) * AH] = (
                r.reshape(F, AH, Nbr).transpose(1, 2, 0))
    return un, ue
